# revision 9
# baseline (speedup 1.0000x reference)
"""Trainium2 Bass kernel for nn_MultiHeadAttention (B=2, S=2048, E=1024, H=8, D=128).

Sharding (8 cores): core c handles batch b=c//4 and head-pair g=c%4
(heads 2g, 2g+1 -> E-columns [256g, 256g+256)).
 - Q/K/V projections column-parallel (each core computes its 256 columns).
 - Attention device-local per head, computed in transposed score layout
   scoresT[k, q] so softmaxed weights are directly the rhs of attn@V.
 - Out-projection row-parallel: each core produces a full-shape partial
   out_partial = attn_out_heads @ Wo[rows] in fp16; host sums 4 partials
   per batch and adds bo.
 - Causal mask realized structurally: fully-masked k-tiles are skipped,
   diagonal k-tiles get width-trimmed score matmuls + ragged exp, the
   128-wide diagonal band is masked by one persistent 0/1 triangle tile,
   and the left strips are zeroed by one memset per (head, q-chunk).
 - Softmax denominator: bf16 pairwise-tree partial sums on DVE, a bf16
   ones-column matmul for the partition sum, reciprocal on DVE, and a
   gpsimd partition_broadcast; bv is applied post-attention as a
   per-partition scalar add (valid because attention rows sum to 1);
   bq/bk are applied on the Act engine during the projections.
 - Weights / biases stay resident in SBUF across chained iterations;
   per-iteration HBM traffic is the three activations (bf16, transposed)
   in and the fp16 partial out.
"""

import os
import sys

for _p in ("/opt/trn_rl_repo", os.environ.get("TRN_RL_REPO", "")):
    if _p and os.path.isdir(_p) and _p not in sys.path:
        sys.path.insert(0, _p)

import numpy as np
import ml_dtypes

BF16 = ml_dtypes.bfloat16
FP16 = np.float16

B, S, E, H = 2, 2048, 1024, 8
D = E // H          # 128
HP = 2              # heads per core
C = HP * D          # 256 projection columns per core
NCORES = 8
KT = S // 128       # 16 k-tiles
QC = S // 512       # 4 q-chunks
SCALE = 1.0 / float(np.sqrt(D))

_prog_cache = {}


def build_program(n_iters: int = 1, **opt):
    """Build the SPMD Bass program (Tile). Returns the compiled Bacc object."""
    import concourse.bass as bass
    import concourse.mybir as mybir
    import concourse.tile as tile
    from concourse import bacc, bass_isa
    from concourse.masks import make_identity
    from contextlib import ExitStack

    f32 = mybir.dt.float32
    bf16 = mybir.dt.bfloat16
    fp16 = mybir.dt.float16
    AF = mybir.ActivationFunctionType

    o = dict(xt_bufs=12, expt_bufs=2, scr_bufs=3, acc_bufs=2, rinv_bufs=2,
             rs_bufs=2, outst_bufs=4, sc_bufs=2, ot_bufs=2, op_bufs=1,
             cs_bufs=1, proj_bufs=4, dbuf=True, use_parred=False)
    o.update(opt)

    nc = bacc.Bacc("TRN2", target_bir_lowering=False, debug=False,
                   enable_partition_id=False)

    # ---- DRAM I/O (per-core slices supplied by the host) ----
    xq_t = nc.dram_tensor("xq_t", [E, S], bf16, kind="ExternalInput")
    xk_t = nc.dram_tensor("xk_t", [E, S], bf16, kind="ExternalInput")
    xv_t = nc.dram_tensor("xv_t", [E, S], bf16, kind="ExternalInput")
    wq_d = nc.dram_tensor("wq", [E, C], bf16, kind="ExternalInput")
    wk_d = nc.dram_tensor("wk", [E, C], bf16, kind="ExternalInput")
    wv_d = nc.dram_tensor("wv", [E, C], bf16, kind="ExternalInput")
    wo_d = nc.dram_tensor("wo", [C, E], bf16, kind="ExternalInput")
    bqk_d = nc.dram_tensor("bqk", [128, 4], f32, kind="ExternalInput")
    bvt_d = nc.dram_tensor("bvt", [128, HP], f32, kind="ExternalInput")
    out_d = nc.dram_tensor("out", [S, E], fp16, kind="ExternalOutput")

    with tile.TileContext(nc) as tc, ExitStack() as ctx:
        persist = ctx.enter_context(tc.tile_pool(name="persist", bufs=1))
        xt_pool = ctx.enter_context(tc.tile_pool(name="xt", bufs=o["xt_bufs"]))
        expt_pool = ctx.enter_context(tc.tile_pool(name="expt",
                                                   bufs=o["expt_bufs"]))
        scr_pool = ctx.enter_context(tc.tile_pool(name="scr",
                                                  bufs=o["scr_bufs"]))
        acc_pool = ctx.enter_context(tc.tile_pool(name="acc",
                                                  bufs=o["acc_bufs"]))
        rinv_pool = ctx.enter_context(tc.tile_pool(name="rinv",
                                                   bufs=o["rinv_bufs"]))
        rs_pool = ctx.enter_context(tc.tile_pool(name="rs", bufs=o["rs_bufs"]))
        outst = ctx.enter_context(tc.tile_pool(name="outst",
                                               bufs=o["outst_bufs"]))
        ps_op = ctx.enter_context(tc.tile_pool(name="ps_op", bufs=o["op_bufs"],
                                               space="PSUM"))
        ps_cs = ctx.enter_context(tc.tile_pool(name="ps_cs", bufs=o["cs_bufs"],
                                               space="PSUM"))

        # ---- constants ----
        ones_col = persist.tile([128, 1], bf16, tag="ones_col")
        nc.vector.memset(ones_col, 1.0)
        # tri01[k, q] = 1 if q >= k else 0 (bf16) for the diagonal band
        tri01 = persist.tile([128, 128], bf16, tag="tri01")
        nc.gpsimd.memset(tri01, 1.0)
        nc.gpsimd.affine_select(out=tri01, in_=tri01,
                                compare_op=mybir.AluOpType.is_ge, fill=0.0,
                                base=0, pattern=[[1, 128]],
                                channel_multiplier=-1)

        # ---- persistent weights / biases (loaded once, resident) ----
        wq_sb = persist.tile([128, 8, C], bf16, tag="wq")
        wk_sb = persist.tile([128, 8, C], bf16, tag="wk")
        wv_sb = persist.tile([128, 8, C], bf16, tag="wv")
        wo_sb = persist.tile([128, HP, E], bf16, tag="wo")
        bqk = persist.tile([128, 4], f32, tag="bqk")
        bvt = persist.tile([128, HP], f32, tag="bvt")
        nc.sync.dma_start(out=wq_sb,
                          in_=wq_d.ap().rearrange("(c p) n -> p c n", p=128))
        nc.sync.dma_start(out=wk_sb,
                          in_=wk_d.ap().rearrange("(c p) n -> p c n", p=128))
        nc.sync.dma_start(out=wv_sb,
                          in_=wv_d.ap().rearrange("(c p) n -> p c n", p=128))
        nc.sync.dma_start(out=wo_sb,
                          in_=wo_d.ap().rearrange("(h p) n -> p h n", p=128))
        nc.sync.dma_start(out=bqk, in_=bqk_d.ap())
        nc.sync.dma_start(out=bvt, in_=bvt_d.ap())

        def make_op_emitter(ot_list):
            def emit_op_chunk(s):
                for nch in range(2):
                    nsl = slice(nch * 512, (nch + 1) * 512)
                    ps = ps_op.tile([128, 512], f32, tag="op")
                    for hh in range(HP):
                        nc.tensor.matmul(
                            ps,
                            lhsT=ot_list[hh][:, s * 128:(s + 1) * 128],
                            rhs=wo_sb[:, hh, nsl],
                            start=(hh == 0), stop=(hh == HP - 1))
                    osb = outst.tile([128, 512], fp16, tag="osb")
                    nc.vector.tensor_copy(osb, ps)
                    nc.sync.dma_start(
                        out=out_d[s * 128:(s + 1) * 128, nsl], in_=osb)
            return emit_op_chunk

        deferred = []   # (emitter, s) pairs carried into the next proj phase

        for it in range(n_iters):
            par = ("a" if it % 2 == 0 else "b") if o["dbuf"] else "a"
            qt_sb = [persist.tile([128, S], bf16, tag=f"qt{m}{par}",
                                  name=f"qt{m}{par}")
                     for m in range(HP)]
            kt_sb = [persist.tile([128, S], bf16, tag=f"kt{m}{par}",
                                  name=f"kt{m}{par}")
                     for m in range(HP)]
            v_sb = persist.tile([128, KT, C], bf16, tag=f"v{par}", name=f"v{par}")
            ot_sb = [persist.tile([128, S], bf16, tag=f"ot{m}{par}",
                                  name=f"ot{m}{par}")
                     for m in range(HP)]

            # ================= Phase 1: projections =================
            # Deferred out-proj chunks from the previous iteration run here,
            # interleaved into the QT matmul stream to fill DMA-wait slack.
            dq = list(deferred)
            deferred = []
            with tc.tile_pool(name="ps_proj", bufs=o["proj_bufs"],
                              space="PSUM") as ps_proj:
                # QT / KT: [C, S] = W.T @ X.T; k-chunk outer, one head-pass
                # at a time (4 PSUM banks each).
                for tname, xdram, wsb, qkts, bcol in (
                    ("q", xq_t, wq_sb, qt_sb, 0),
                    ("k", xk_t, wk_sb, kt_sb, 2),
                ):
                    xcs = []
                    for c in range(8):
                        xc = xt_pool.tile([128, S], bf16, tag="xtc")
                        nc.sync.dma_start(
                            out=xc, in_=xdram[c * 128:(c + 1) * 128, :])
                        xcs.append(xc)
                    for m in range(HP):
                        pss = [ps_proj.tile([128, 512], f32, tag="ps_proj",
                                            name=f"ps_{tname}{m}{_i}")
                               for _i in range(QC)]
                        for c in range(8):
                            for n in range(QC):
                                nc.tensor.matmul(
                                    pss[n],
                                    lhsT=wsb[:, c, m * 128:(m + 1) * 128],
                                    rhs=xcs[c][:, n * 512:(n + 1) * 512],
                                    start=(c == 0), stop=(c == 7))
                            if tname == "q" and m == 0 and dq:
                                em, s_ = dq.pop(0)
                                em(s_)
                        for n in range(QC):
                            nc.scalar.activation(
                                out=qkts[m][:, n * 512:(n + 1) * 512],
                                in_=pss[n],
                                func=AF.Identity,
                                bias=bqk[:, bcol + m:bcol + m + 1], scale=1.0)
                while dq:
                    em, s_ = dq.pop(0)
                    em(s_)

                # V natural: [S, C] = X @ Wv (lhsT = XT chunk slice); psum ->
                # SBUF copy on the Act engine (no bias: bv applied later).
                xcs = []
                for c in range(8):
                    xc = xt_pool.tile([128, S], bf16, tag="xtc")
                    nc.sync.dma_start(out=xc, in_=xv_t[c * 128:(c + 1) * 128, :])
                    xcs.append(xc)
                for s in range(KT):
                    ps = ps_proj.tile([128, C], f32, tag="ps_proj")
                    for c in range(8):
                        nc.tensor.matmul(
                            ps,
                            lhsT=xcs[c][:, s * 128:(s + 1) * 128],
                            rhs=wv_sb[:, c, :],
                            start=(c == 0), stop=(c == 7))
                    nc.scalar.activation(out=v_sb[:, s, :], in_=ps,
                                         func=AF.Copy)

            # ================= Phase 2: attention + fused out-proj ========
            with tc.tile_pool(name="ps_sc", bufs=o["sc_bufs"],
                              space="PSUM") as ps_sc, \
                 tc.tile_pool(name="ps_ot", bufs=o["ot_bufs"],
                              space="PSUM") as ps_ot:

                emit_op_chunk = make_op_emitter(ot_sb)

                # big-j first; tiny-j steps are sandwiched between large-PE
                # steps so their softmax chains hide under PE work.
                HJ = [(0, 3), (1, 3), (0, 0), (0, 2), (1, 2), (1, 0),
                      (0, 1), (1, 1)]
                # out-proj schedule (2 seq-tiles per step, one group behind);
                # groups not ready until late are deferred into the next
                # iteration's projection phase.
                OPS = {3: [12, 13], 4: [14, 15], 5: [8, 9], 6: [10, 11],
                       7: [0, 1]}
                DEFER = [2, 3, 4, 5, 6, 7]

                for i, (h, j) in enumerate(HJ):
                    nk = 4 * (j + 1)
                    qsl = slice(j * 512, (j + 1) * 512)
                    et = expt_pool.tile([128, KT, 512], bf16, tag="et")
                    # zero the 3 ragged diagonal tiles (full width keeps the
                    # access pattern contiguous -> fast DVE mode; the exp
                    # writes below overwrite the valid right-hand regions)
                    nc.gpsimd.memset(et[:, 4 * j + 1:4 * j + 4, :], 0.0)

                    # off-diagonal k-tiles, processed in pairs (one wide
                    # exp); the off-diagonal part of the denominator tree is
                    # emitted as soon as its inputs exist so it hides under
                    # the Act-paced score stream.
                    offp = []   # pending off-diag partial tiles to combine
                    for p in range(2 * j):
                        ps2 = ps_sc.tile([128, 2, 512], f32, tag="sc2")
                        for u in range(2):
                            kti = 2 * p + u
                            nc.tensor.matmul(
                                ps2[:, u, :],
                                lhsT=kt_sb[h][:, kti * 128:(kti + 1) * 128],
                                rhs=qt_sb[h][:, qsl],
                                start=True, stop=True)
                        nc.scalar.activation(
                            out=et[:, 2 * p:2 * p + 2, :], in_=ps2,
                            func=AF.Exp, scale=SCALE)
                        if p % 2 == 1:
                            t4 = scr_pool.tile([128, 4, 512], bf16, tag="scr",
                                               name=f"scr{i}_{p}")
                            nc.vector.tensor_add(
                                t4[:, 0:2, :], et[:, 2 * p - 2:2 * p, :],
                                et[:, 2 * p:2 * p + 2, :])
                            nc.vector.tensor_add(t4[:, 0, :], t4[:, 0, :],
                                                 t4[:, 1, :])
                            offp.append(t4)

                    # diagonal k-tiles: width-trimmed scores + ragged exp +
                    # triangle-band mask
                    for dp in range(2):
                        ps2 = ps_sc.tile([128, 2, 512], f32, tag="sc2")
                        for u in range(2):
                            t = 2 * dp + u
                            w0 = 128 * t
                            nc.tensor.matmul(
                                ps2[:, u, w0:512],
                                lhsT=kt_sb[h][:, (4 * j + t) * 128:
                                              (4 * j + t + 1) * 128],
                                rhs=qt_sb[h][:, j * 512 + w0:(j + 1) * 512],
                                start=True, stop=True)
                        for u in range(2):
                            t = 2 * dp + u
                            w0 = 128 * t
                            nc.scalar.activation(
                                out=et[:, 4 * j + t, w0:512],
                                in_=ps2[:, u, w0:512],
                                func=AF.Exp, scale=SCALE)
                        for u in range(2):
                            t = 2 * dp + u
                            w0 = 128 * t
                            nc.vector.tensor_mul(
                                et[:, 4 * j + t, w0:w0 + 128],
                                et[:, 4 * j + t, w0:w0 + 128], tri01)

                    # attn @ V -> ot[d, q] accumulated over k-tiles; the
                    # diagonal tiles only contribute to q >= 128t, so trim
                    # the moving operand accordingly.
                    ot = ps_ot.tile([128, 512], f32, tag="ot")
                    for kti in range(nk):
                        w0 = 128 * (kti - 4 * j) if kti >= 4 * j else 0
                        nc.tensor.matmul(
                            ot[:, w0:512],
                            lhsT=v_sb[:, kti, h * 128:(h + 1) * 128],
                            rhs=et[:, kti, w0:512],
                            start=(kti == 0), stop=(kti == nk - 1))

                    # denominator: diag part (short critical path) + the
                    # pre-computed off-diag partials
                    accum = acc_pool.tile([128, 512], bf16, tag="acc")
                    dt = scr_pool.tile([128, 2, 512], bf16, tag="scrd")
                    nc.vector.tensor_add(dt, et[:, 4 * j:4 * j + 2, :],
                                         et[:, 4 * j + 2:4 * j + 4, :])
                    if not offp:
                        nc.vector.tensor_add(accum, dt[:, 0, :], dt[:, 1, :])
                    else:
                        nc.vector.tensor_add(dt[:, 0, :], dt[:, 0, :],
                                             dt[:, 1, :])
                        if len(offp) == 1:
                            nc.vector.tensor_add(accum, dt[:, 0, :],
                                                 offp[0][:, 0, :])
                        elif len(offp) == 2:
                            nc.vector.tensor_add(dt[:, 1, :],
                                                 offp[0][:, 0, :],
                                                 offp[1][:, 0, :])
                            nc.vector.tensor_add(accum, dt[:, 0, :],
                                                 dt[:, 1, :])
                        else:  # 3 partials (j == 3)
                            nc.vector.tensor_add(dt[:, 1, :],
                                                 offp[0][:, 0, :],
                                                 offp[1][:, 0, :])
                            nc.vector.tensor_add(dt[:, 0, :], dt[:, 0, :],
                                                 offp[2][:, 0, :])
                            nc.vector.tensor_add(accum, dt[:, 0, :],
                                                 dt[:, 1, :])

                    # partition sum via bf16 ones matmul; reciprocal;
                    # broadcast; normalize + bv
                    rs = rs_pool.tile([128, 512], f32, tag="rs")
                    if o["use_parred"]:
                        nc.gpsimd.partition_all_reduce(
                            rs, accum, channels=128,
                            reduce_op=bass_isa.ReduceOp.add)
                        nc.vector.reciprocal(rs, rs)
                    else:
                        cs = ps_cs.tile([1, 512], f32, tag="cs")
                        nc.tensor.matmul(cs, lhsT=ones_col, rhs=accum,
                                         start=True, stop=True)
                        rin = rinv_pool.tile([1, 512], f32, tag="rinv")
                        nc.vector.reciprocal(rin, cs)
                        nc.gpsimd.partition_broadcast(rs, rin)
                    nc.vector.tensor_mul(ot_sb[h][:, qsl], ot, rs)
                    nc.vector.tensor_scalar_add(ot_sb[h][:, qsl],
                                                ot_sb[h][:, qsl],
                                                bvt[:, h:h + 1])

                    for s in OPS.get(i, []):
                        emit_op_chunk(s)

                for s in DEFER:
                    deferred.append((emit_op_chunk, s))

        # drain any deferred out-proj chunks of the final iteration
        for em, s_ in deferred:
            em(s_)

    nc.compile()
    return nc


def get_program(n_iters: int = 1):
    if n_iters not in _prog_cache:
        _prog_cache[n_iters] = build_program(n_iters)
    return _prog_cache[n_iters]


def make_in_maps(query, key_, value, Wq, bq, Wk, bk, Wv, bv, Wo, bo, mask):
    """Host-side sharding: build the 8 per-core input maps."""
    query = np.asarray(query, np.float32)
    key_ = np.asarray(key_, np.float32)
    value = np.asarray(value, np.float32)

    # transposed bf16 activations per batch: [E, S]
    xt = {}
    for b in range(B):
        xt[("q", b)] = np.ascontiguousarray(query[b].T.astype(BF16))
        xt[("k", b)] = np.ascontiguousarray(key_[b].T.astype(BF16))
        xt[("v", b)] = np.ascontiguousarray(value[b].T.astype(BF16))

    Wq = np.asarray(Wq, np.float32)
    Wk = np.asarray(Wk, np.float32)
    Wv = np.asarray(Wv, np.float32)
    Wo = np.asarray(Wo, np.float32)
    bq = np.asarray(bq, np.float32)
    bk = np.asarray(bk, np.float32)
    bv = np.asarray(bv, np.float32)

    in_maps = []
    for c in range(NCORES):
        b, g = divmod(c, 4)
        c0 = C * g
        bqk = np.stack([bq[c0:c0 + 128], bq[c0 + 128:c0 + 256],
                        bk[c0:c0 + 128], bk[c0 + 128:c0 + 256]], axis=1)
        bvt = np.stack([bv[c0:c0 + 128], bv[c0 + 128:c0 + 256]], axis=1)
        in_maps.append({
            "xq_t": xt[("q", b)],
            "xk_t": xt[("k", b)],
            "xv_t": xt[("v", b)],
            "wq": Wq[:, c0:c0 + C].astype(BF16),
            "wk": Wk[:, c0:c0 + C].astype(BF16),
            "wv": Wv[:, c0:c0 + C].astype(BF16),
            "wo": np.ascontiguousarray(Wo[c0:c0 + C, :]).astype(BF16),
            "bqk": np.ascontiguousarray(bqk, dtype=np.float32),
            "bvt": np.ascontiguousarray(bvt, dtype=np.float32),
        })
    return in_maps


_bo_cache = {}


def gather_output(results, bo):
    out = np.zeros((B, S, E), np.float32)
    for c in range(NCORES):
        b = c // 4
        out[b] += results[c]["out"].astype(np.float32)
    out += np.asarray(bo, np.float32)
    return out


def kernel(**inputs) -> np.ndarray:
    from concourse.bass_utils import run_bass_kernel_spmd

    nc = get_program(1)
    in_maps = make_in_maps(**inputs)
    res = run_bass_kernel_spmd(nc, in_maps, core_ids=list(range(NCORES)))
    return gather_output(res.results, inputs["bo"])


# revision 16
# speedup vs baseline: 1.3692x; 1.3692x over previous
"""Trainium2 Bass kernel for nn_MultiHeadAttention (B=2, S=2048, E=1024, H=8, D=128).

Sharding (8 cores): core c handles batch b=c//4 and head-pair g=c%4
(heads 2g, 2g+1 -> E-columns [256g, 256g+256)).
 - Q/K/V projections column-parallel (each core computes its 256 columns).
 - Attention device-local per head, computed in transposed score layout
   scoresT[k, q] so softmaxed weights are directly the rhs of attn@V.
 - Out-projection row-parallel: each core produces a full-shape partial
   out_partial = attn_out_heads @ Wo[rows] in fp16; host sums 4 partials
   per batch and adds bo.
 - Causal mask realized structurally: fully-masked k-tiles are skipped,
   diagonal k-tiles get width-trimmed score matmuls + ragged exp, the
   128-wide diagonal band is masked by one persistent 0/1 triangle tile,
   and the left strips are zeroed by one memset per (head, q-chunk).
 - Softmax denominator: bf16 pairwise-tree partial sums on DVE, a bf16
   ones-column matmul for the partition sum, reciprocal on DVE, and a
   gpsimd partition_broadcast; bv is applied post-attention as a
   per-partition scalar add (valid because attention rows sum to 1);
   bq/bk are applied on the Act engine during the projections.
 - Weights / biases stay resident in SBUF across chained iterations;
   per-iteration HBM traffic is the three activations (bf16, transposed)
   in and the fp16 partial out.
"""

import os
import sys

for _p in ("/opt/trn_rl_repo", os.environ.get("TRN_RL_REPO", "")):
    if _p and os.path.isdir(_p) and _p not in sys.path:
        sys.path.insert(0, _p)

import numpy as np
import ml_dtypes

BF16 = ml_dtypes.bfloat16
FP16 = np.float16

B, S, E, H = 2, 2048, 1024, 8
D = E // H          # 128
HP = 2              # heads per core
C = HP * D          # 256 projection columns per core
NCORES = 8
KT = S // 128       # 16 k-tiles
QC = S // 512       # 4 q-chunks
SCALE = 1.0 / float(np.sqrt(D))

_prog_cache = {}


def build_program(n_iters: int = 1, **opt):
    """Build the SPMD Bass program (Tile). Returns the compiled Bacc object."""
    import concourse.bass as bass
    import concourse.mybir as mybir
    import concourse.tile as tile
    from concourse import bacc, bass_isa
    from concourse.masks import make_identity
    from contextlib import ExitStack

    f32 = mybir.dt.float32
    bf16 = mybir.dt.bfloat16
    fp16 = mybir.dt.float16
    AF = mybir.ActivationFunctionType

    o = dict(xt_bufs=14, expt_bufs=2, scr_bufs=3, acc_bufs=3, rinv_bufs=2,
             rs_bufs=2, outst_bufs=6, sc_bufs=2, ot_bufs=2, op_bufs=1,
             cs_bufs=1, proj_bufs=4, dbuf=True, use_parred=False)
    o.update(opt)

    nc = bacc.Bacc("TRN2", target_bir_lowering=False, debug=False,
                   enable_partition_id=False)

    # ---- DRAM I/O (per-core slices supplied by the host) ----
    xq_t = nc.dram_tensor("xq_t", [E, S], bf16, kind="ExternalInput")
    xk_t = nc.dram_tensor("xk_t", [E, S], bf16, kind="ExternalInput")
    xv_t = nc.dram_tensor("xv_t", [E, S], bf16, kind="ExternalInput")
    wq_d = nc.dram_tensor("wq", [E, C], bf16, kind="ExternalInput")
    wk_d = nc.dram_tensor("wk", [E, C], bf16, kind="ExternalInput")
    wv_d = nc.dram_tensor("wv", [E, C], bf16, kind="ExternalInput")
    wo_d = nc.dram_tensor("wo", [C, E], bf16, kind="ExternalInput")
    bqk_d = nc.dram_tensor("bqk", [128, 4], f32, kind="ExternalInput")
    bvt_d = nc.dram_tensor("bvt", [128, HP], f32, kind="ExternalInput")
    out_d = nc.dram_tensor("out", [S, E], fp16, kind="ExternalOutput")

    with tile.TileContext(nc) as tc, ExitStack() as ctx:
        persist = ctx.enter_context(tc.tile_pool(name="persist", bufs=1))
        xt_pool = ctx.enter_context(tc.tile_pool(name="xt", bufs=o["xt_bufs"]))
        expt_pool = ctx.enter_context(tc.tile_pool(name="expt",
                                                   bufs=o["expt_bufs"]))
        scr_pool = ctx.enter_context(tc.tile_pool(name="scr",
                                                  bufs=o["scr_bufs"]))
        acc_pool = ctx.enter_context(tc.tile_pool(name="acc",
                                                  bufs=o["acc_bufs"]))
        rinv_pool = ctx.enter_context(tc.tile_pool(name="rinv",
                                                   bufs=o["rinv_bufs"]))
        rs_pool = ctx.enter_context(tc.tile_pool(name="rs", bufs=o["rs_bufs"]))
        outst = ctx.enter_context(tc.tile_pool(name="outst",
                                               bufs=o["outst_bufs"]))
        ps_op = ctx.enter_context(tc.tile_pool(name="ps_op", bufs=o["op_bufs"],
                                               space="PSUM"))
        ps_cs = ctx.enter_context(tc.tile_pool(name="ps_cs", bufs=o["cs_bufs"],
                                               space="PSUM"))

        # ---- constants ----
        ones_col = persist.tile([128, 1], bf16, tag="ones_col")
        nc.vector.memset(ones_col, 1.0)
        # tri01[k, q] = 1 if q >= k else 0 (bf16) for the diagonal band
        tri01 = persist.tile([128, 128], bf16, tag="tri01")
        nc.gpsimd.memset(tri01, 1.0)
        nc.gpsimd.affine_select(out=tri01, in_=tri01,
                                compare_op=mybir.AluOpType.is_ge, fill=0.0,
                                base=0, pattern=[[1, 128]],
                                channel_multiplier=-1)

        # ---- persistent weights / biases (loaded once, resident) ----
        wq_sb = persist.tile([128, 8, C], bf16, tag="wq")
        wk_sb = persist.tile([128, 8, C], bf16, tag="wk")
        wv_sb = persist.tile([128, 8, C], bf16, tag="wv")
        wo_sb = persist.tile([128, HP, E], bf16, tag="wo")
        bqk = persist.tile([128, 4], f32, tag="bqk")
        bvt = persist.tile([128, HP], f32, tag="bvt")
        nc.sync.dma_start(out=wq_sb,
                          in_=wq_d.ap().rearrange("(c p) n -> p c n", p=128))
        nc.sync.dma_start(out=wk_sb,
                          in_=wk_d.ap().rearrange("(c p) n -> p c n", p=128))
        nc.sync.dma_start(out=wv_sb,
                          in_=wv_d.ap().rearrange("(c p) n -> p c n", p=128))
        nc.sync.dma_start(out=wo_sb,
                          in_=wo_d.ap().rearrange("(h p) n -> p h n", p=128))
        nc.sync.dma_start(out=bqk, in_=bqk_d.ap())
        nc.sync.dma_start(out=bvt, in_=bvt_d.ap())

        def make_op_emitter(ot_list):
            def emit_op_chunk(s, on_act=False):
                for nch in range(2):
                    nsl = slice(nch * 512, (nch + 1) * 512)
                    ps = ps_op.tile([128, 512], f32, tag="op")
                    for hh in range(HP):
                        nc.tensor.matmul(
                            ps,
                            lhsT=ot_list[hh][:, s * 128:(s + 1) * 128],
                            rhs=wo_sb[:, hh, nsl],
                            start=(hh == 0), stop=(hh == HP - 1))
                    osb = outst.tile([128, 512], fp16, tag="osb")
                    if on_act:
                        nc.scalar.activation(out=osb, in_=ps, func=AF.Copy)
                    else:
                        nc.vector.tensor_copy(osb, ps)
                    nc.sync.dma_start(
                        out=out_d[s * 128:(s + 1) * 128, nsl], in_=osb)
            return emit_op_chunk

        deferred = []   # (emitter, s) pairs carried into the next proj phase

        for it in range(n_iters):
            par = ("a" if it % 2 == 0 else "b") if o["dbuf"] else "a"
            qt_sb = [persist.tile([128, S], bf16, tag=f"qt{m}{par}",
                                  name=f"qt{m}{par}")
                     for m in range(HP)]
            kt_sb = [persist.tile([128, S], bf16, tag=f"kt{m}{par}",
                                  name=f"kt{m}{par}")
                     for m in range(HP)]
            v_sb = persist.tile([128, KT, C], bf16, tag=f"v{par}", name=f"v{par}")
            ot_sb = [persist.tile([128, S], bf16, tag=f"ot{m}{par}",
                                  name=f"ot{m}{par}")
                     for m in range(HP)]

            # ================= Phase 1: projections =================
            # Deferred out-proj chunks from the previous iteration run here,
            # interleaved into the QT matmul stream to fill DMA-wait slack.
            dq = list(deferred)
            deferred = []
            with tc.tile_pool(name="ps_proj", bufs=o["proj_bufs"],
                              space="PSUM") as ps_proj:
                # QT / KT: [C, S] = W.T @ X.T; k-chunk outer, one head-pass
                # at a time (4 PSUM banks each).
                for tname, xdram, wsb, qkts, bcol in (
                    ("q", xq_t, wq_sb, qt_sb, 0),
                    ("k", xk_t, wk_sb, kt_sb, 2),
                ):
                    xcs = []
                    for c in range(8):
                        xc = xt_pool.tile([128, S], bf16, tag="xtc")
                        nc.sync.dma_start(
                            out=xc, in_=xdram[c * 128:(c + 1) * 128, :])
                        xcs.append(xc)
                    for m in range(HP):
                        pss = [ps_proj.tile([128, 512], f32, tag="ps_proj",
                                            name=f"ps_{tname}{m}{_i}")
                               for _i in range(QC)]
                        for c in range(8):
                            for n in range(QC):
                                nc.tensor.matmul(
                                    pss[n],
                                    lhsT=wsb[:, c, m * 128:(m + 1) * 128],
                                    rhs=xcs[c][:, n * 512:(n + 1) * 512],
                                    start=(c == 0), stop=(c == 7))
                            if m == 0 and dq and (
                                    tname == "q" or o.get("defer_more")):
                                em, s_ = dq.pop(0)
                                em(s_)
                        for n in range(QC):
                            nc.scalar.activation(
                                out=qkts[m][:, n * 512:(n + 1) * 512],
                                in_=pss[n],
                                func=AF.Identity,
                                bias=bqk[:, bcol + m:bcol + m + 1], scale=1.0)
                while dq:
                    em, s_ = dq.pop(0)
                    em(s_)

                # V natural: [S, C] = X @ Wv (lhsT = XT chunk slice); psum ->
                # SBUF copy on the Act engine (no bias: bv applied later).
                xcs = []
                for c in range(8):
                    xc = xt_pool.tile([128, S], bf16, tag="xtc")
                    nc.sync.dma_start(out=xc, in_=xv_t[c * 128:(c + 1) * 128, :])
                    xcs.append(xc)
                for s in range(KT):
                    ps = ps_proj.tile([128, C], f32, tag="ps_proj")
                    for c in range(8):
                        nc.tensor.matmul(
                            ps,
                            lhsT=xcs[c][:, s * 128:(s + 1) * 128],
                            rhs=wv_sb[:, c, :],
                            start=(c == 0), stop=(c == 7))
                    nc.scalar.activation(out=v_sb[:, s, :], in_=ps,
                                         func=AF.Copy)

            # ================= Phase 2: attention + fused out-proj ========
            with tc.tile_pool(name="ps_sc", bufs=o["sc_bufs"],
                              space="PSUM") as ps_sc, \
                 tc.tile_pool(name="ps_ot", bufs=o["ot_bufs"],
                              space="PSUM") as ps_ot:

                emit_op_chunk = make_op_emitter(ot_sb)

                # big-j first; tiny-j steps are sandwiched between large-PE
                # steps so their softmax chains hide under PE work.
                HJ = [(0, 3), (1, 3), (0, 0), (0, 2), (1, 2), (1, 0),
                      (0, 1), (1, 1)]
                # out-proj schedule (2 seq-tiles per step, one group behind);
                # groups not ready until late are deferred into the next
                # iteration's projection phase.
                if o.get("defer_more"):
                    OPS = {5: [8, 9], 6: [10, 11], 7: [0, 1]}
                    DEFER = [12, 13, 14, 15, 2, 3, 4, 5, 6, 7]
                else:
                    OPS = {3: [12, 13], 4: [14, 15], 5: [8, 9], 6: [10, 11],
                           7: [0, 1]}
                    DEFER = [2, 3, 4, 5, 6, 7]

                for i, (h, j) in enumerate(HJ):
                    nk = 4 * (j + 1)
                    qsl = slice(j * 512, (j + 1) * 512)
                    et = expt_pool.tile([128, KT, 512], bf16, tag="et")
                    # zero the 3 ragged diagonal tiles (full width keeps the
                    # access pattern contiguous -> fast DVE mode; the exp
                    # writes below overwrite the valid right-hand regions)
                    nc.gpsimd.memset(et[:, 4 * j + 1:4 * j + 4, :], 0.0)

                    # off-diagonal k-tiles, processed in pairs (one wide
                    # exp); the off-diagonal part of the denominator tree is
                    # emitted as soon as its inputs exist so it hides under
                    # the Act-paced score stream.
                    offp = []   # pending off-diag partial tiles to combine
                    for p in range(2 * j):
                        ps2 = ps_sc.tile([128, 2, 512], f32, tag="sc2")
                        for u in range(2):
                            kti = 2 * p + u
                            nc.tensor.matmul(
                                ps2[:, u, :],
                                lhsT=kt_sb[h][:, kti * 128:(kti + 1) * 128],
                                rhs=qt_sb[h][:, qsl],
                                start=True, stop=True)
                        nc.scalar.activation(
                            out=et[:, 2 * p:2 * p + 2, :], in_=ps2,
                            func=AF.Exp, scale=SCALE)
                        if p % 2 == 1:
                            t4 = scr_pool.tile([128, 4, 512], bf16, tag="scr",
                                               name=f"scr{i}_{p}")
                            nc.vector.tensor_add(
                                t4[:, 0:2, :], et[:, 2 * p - 2:2 * p, :],
                                et[:, 2 * p:2 * p + 2, :])
                            nc.vector.tensor_add(t4[:, 0, :], t4[:, 0, :],
                                                 t4[:, 1, :])
                            offp.append(t4)

                    # diagonal k-tiles: width-trimmed scores + ragged exp +
                    # triangle-band mask
                    for dp in range(2):
                        ps2 = ps_sc.tile([128, 2, 512], f32, tag="sc2")
                        for u in range(2):
                            t = 2 * dp + u
                            w0 = 128 * t
                            nc.tensor.matmul(
                                ps2[:, u, w0:512],
                                lhsT=kt_sb[h][:, (4 * j + t) * 128:
                                              (4 * j + t + 1) * 128],
                                rhs=qt_sb[h][:, j * 512 + w0:(j + 1) * 512],
                                start=True, stop=True)
                        for u in range(2):
                            t = 2 * dp + u
                            w0 = 128 * t
                            nc.scalar.activation(
                                out=et[:, 4 * j + t, w0:512],
                                in_=ps2[:, u, w0:512],
                                func=AF.Exp, scale=SCALE)
                        for u in range(2):
                            t = 2 * dp + u
                            w0 = 128 * t
                            nc.vector.tensor_mul(
                                et[:, 4 * j + t, w0:w0 + 128],
                                et[:, 4 * j + t, w0:w0 + 128], tri01)

                    # attn @ V -> ot[d, q] accumulated over k-tiles; the
                    # diagonal tiles only contribute to q >= 128t, so trim
                    # the moving operand accordingly.
                    ot = ps_ot.tile([128, 512], f32, tag="ot")
                    for kti in range(nk):
                        w0 = 128 * (kti - 4 * j) if kti >= 4 * j else 0
                        nc.tensor.matmul(
                            ot[:, w0:512],
                            lhsT=v_sb[:, kti, h * 128:(h + 1) * 128],
                            rhs=et[:, kti, w0:512],
                            start=(kti == 0), stop=(kti == nk - 1))

                    # denominator: diag part (short critical path) + the
                    # pre-computed off-diag partials
                    accum = acc_pool.tile([128, 512], bf16, tag="acc")
                    dt = scr_pool.tile([128, 2, 512], bf16, tag="scrd")
                    nc.vector.tensor_add(dt, et[:, 4 * j:4 * j + 2, :],
                                         et[:, 4 * j + 2:4 * j + 4, :])
                    if not offp:
                        nc.vector.tensor_add(accum, dt[:, 0, :], dt[:, 1, :])
                    else:
                        nc.vector.tensor_add(dt[:, 0, :], dt[:, 0, :],
                                             dt[:, 1, :])
                        if len(offp) == 1:
                            nc.vector.tensor_add(accum, dt[:, 0, :],
                                                 offp[0][:, 0, :])
                        elif len(offp) == 2:
                            nc.vector.tensor_add(dt[:, 1, :],
                                                 offp[0][:, 0, :],
                                                 offp[1][:, 0, :])
                            nc.vector.tensor_add(accum, dt[:, 0, :],
                                                 dt[:, 1, :])
                        else:  # 3 partials (j == 3)
                            nc.vector.tensor_add(dt[:, 1, :],
                                                 offp[0][:, 0, :],
                                                 offp[1][:, 0, :])
                            nc.vector.tensor_add(dt[:, 0, :], dt[:, 0, :],
                                                 offp[2][:, 0, :])
                            nc.vector.tensor_add(accum, dt[:, 0, :],
                                                 dt[:, 1, :])

                    # partition sum via bf16 ones matmul; reciprocal;
                    # broadcast; normalize + bv
                    rs = rs_pool.tile([128, 512], f32, tag="rs")
                    if o["use_parred"]:
                        nc.gpsimd.partition_all_reduce(
                            rs, accum, channels=128,
                            reduce_op=bass_isa.ReduceOp.add)
                        nc.vector.reciprocal(rs, rs)
                    else:
                        cs = ps_cs.tile([1, 512], f32, tag="cs")
                        nc.tensor.matmul(cs, lhsT=ones_col, rhs=accum,
                                         start=True, stop=True)
                        rin = rinv_pool.tile([1, 512], f32, tag="rinv")
                        nc.vector.reciprocal(rin, cs)
                        nc.gpsimd.partition_broadcast(rs, rin)
                    nc.vector.tensor_mul(ot_sb[h][:, qsl], ot, rs)
                    nc.vector.tensor_scalar_add(ot_sb[h][:, qsl],
                                                ot_sb[h][:, qsl],
                                                bvt[:, h:h + 1])

                    for s in OPS.get(i, []):
                        emit_op_chunk(s)

                for s in DEFER:
                    deferred.append((emit_op_chunk, s))

        # drain any deferred out-proj chunks of the final iteration
        for em, s_ in deferred:
            em(s_)

    nc.compile()
    return nc


def get_program(n_iters: int = 1):
    if n_iters not in _prog_cache:
        _prog_cache[n_iters] = build_program(n_iters)
    return _prog_cache[n_iters]


def make_in_maps(query, key_, value, Wq, bq, Wk, bk, Wv, bv, Wo, bo, mask):
    """Host-side sharding: build the 8 per-core input maps."""
    query = np.asarray(query, np.float32)
    key_ = np.asarray(key_, np.float32)
    value = np.asarray(value, np.float32)

    # transposed bf16 activations per batch: [E, S]
    xt = {}
    for b in range(B):
        xt[("q", b)] = np.ascontiguousarray(query[b].T.astype(BF16))
        xt[("k", b)] = np.ascontiguousarray(key_[b].T.astype(BF16))
        xt[("v", b)] = np.ascontiguousarray(value[b].T.astype(BF16))

    Wq = np.asarray(Wq, np.float32)
    Wk = np.asarray(Wk, np.float32)
    Wv = np.asarray(Wv, np.float32)
    Wo = np.asarray(Wo, np.float32)
    bq = np.asarray(bq, np.float32)
    bk = np.asarray(bk, np.float32)
    bv = np.asarray(bv, np.float32)

    in_maps = []
    for c in range(NCORES):
        b, g = divmod(c, 4)
        c0 = C * g
        bqk = np.stack([bq[c0:c0 + 128], bq[c0 + 128:c0 + 256],
                        bk[c0:c0 + 128], bk[c0 + 128:c0 + 256]], axis=1)
        bvt = np.stack([bv[c0:c0 + 128], bv[c0 + 128:c0 + 256]], axis=1)
        in_maps.append({
            "xq_t": xt[("q", b)],
            "xk_t": xt[("k", b)],
            "xv_t": xt[("v", b)],
            "wq": Wq[:, c0:c0 + C].astype(BF16),
            "wk": Wk[:, c0:c0 + C].astype(BF16),
            "wv": Wv[:, c0:c0 + C].astype(BF16),
            "wo": np.ascontiguousarray(Wo[c0:c0 + C, :]).astype(BF16),
            "bqk": np.ascontiguousarray(bqk, dtype=np.float32),
            "bvt": np.ascontiguousarray(bvt, dtype=np.float32),
        })
    return in_maps


_bo_cache = {}


def gather_output(results, bo):
    out = np.zeros((B, S, E), np.float32)
    for c in range(NCORES):
        b = c // 4
        out[b] += results[c]["out"].astype(np.float32)
    out += np.asarray(bo, np.float32)
    return out


def kernel(**inputs) -> np.ndarray:
    from concourse.bass_utils import run_bass_kernel_spmd

    nc = get_program(1)
    in_maps = make_in_maps(**inputs)
    res = run_bass_kernel_spmd(nc, in_maps, core_ids=list(range(NCORES)))
    return gather_output(res.results, inputs["bo"])


# revision 17
# speedup vs baseline: 1.5347x; 1.1209x over previous
"""Trainium2 Bass kernel for nn_MultiHeadAttention (B=2, S=2048, E=1024, H=8, D=128).

Sharding (8 cores): core c handles batch b=c//4 and head-pair g=c%4
(heads 2g, 2g+1 -> E-columns [256g, 256g+256)).
 - Q/K/V projections column-parallel (each core computes its 256 columns).
 - Attention device-local per head, computed in transposed score layout
   scoresT[k, q] so softmaxed weights are directly the rhs of attn@V.
 - Out-projection row-parallel: each core produces a full-shape partial
   out_partial = attn_out_heads @ Wo[rows] in fp16; host sums 4 partials
   per batch and adds bo.
 - Causal mask realized structurally: fully-masked k-tiles are skipped,
   diagonal k-tiles get width-trimmed score matmuls + ragged exp, the
   128-wide diagonal band is masked by one persistent 0/1 triangle tile,
   and the left strips are zeroed by one memset per (head, q-chunk).
 - Softmax denominator: bf16 pairwise-tree partial sums on DVE, a bf16
   ones-column matmul for the partition sum, reciprocal on DVE, and a
   gpsimd partition_broadcast; bv is applied post-attention as a
   per-partition scalar add (valid because attention rows sum to 1);
   bq/bk are applied on the Act engine during the projections.
 - Weights / biases stay resident in SBUF across chained iterations;
   per-iteration HBM traffic is the three activations (bf16, transposed)
   in and the fp16 partial out.
"""

import os
import sys

for _p in ("/opt/trn_rl_repo", os.environ.get("TRN_RL_REPO", "")):
    if _p and os.path.isdir(_p) and _p not in sys.path:
        sys.path.insert(0, _p)

import numpy as np
import ml_dtypes

BF16 = ml_dtypes.bfloat16
FP16 = np.float16

B, S, E, H = 2, 2048, 1024, 8
D = E // H          # 128
HP = 2              # heads per core
C = HP * D          # 256 projection columns per core
NCORES = 8
KT = S // 128       # 16 k-tiles
QC = S // 512       # 4 q-chunks
SCALE = 1.0 / float(np.sqrt(D))

_prog_cache = {}


def build_program(n_iters: int = 1, **opt):
    """Build the SPMD Bass program (Tile). Returns the compiled Bacc object."""
    import concourse.bass as bass
    import concourse.mybir as mybir
    import concourse.tile as tile
    from concourse import bacc, bass_isa
    from concourse.masks import make_identity
    from contextlib import ExitStack

    f32 = mybir.dt.float32
    bf16 = mybir.dt.bfloat16
    fp16 = mybir.dt.float16
    AF = mybir.ActivationFunctionType

    o = dict(xt_bufs=14, expt_bufs=2, scr_bufs=3, acc_bufs=3, rinv_bufs=2,
             rs_bufs=2, outst_bufs=6, sc_bufs=2, ot_bufs=2, op_bufs=1,
             cs_bufs=1, proj_bufs=4, dbuf=True, use_parred=False)
    o.update(opt)

    nc = bacc.Bacc("TRN2", target_bir_lowering=False, debug=False,
                   enable_partition_id=False)

    # ---- DRAM I/O (per-core slices supplied by the host) ----
    xq_t = nc.dram_tensor("xq_t", [E, S], bf16, kind="ExternalInput")
    xk_t = nc.dram_tensor("xk_t", [E, S], bf16, kind="ExternalInput")
    xv_t = nc.dram_tensor("xv_t", [E, S], bf16, kind="ExternalInput")
    wq_d = nc.dram_tensor("wq", [E, C], bf16, kind="ExternalInput")
    wk_d = nc.dram_tensor("wk", [E, C], bf16, kind="ExternalInput")
    wv_d = nc.dram_tensor("wv", [E, C], bf16, kind="ExternalInput")
    wo_d = nc.dram_tensor("wo", [C, E], bf16, kind="ExternalInput")
    bqk_d = nc.dram_tensor("bqk", [128, 4], f32, kind="ExternalInput")
    bvt_d = nc.dram_tensor("bvt", [128, HP], f32, kind="ExternalInput")
    out_d = nc.dram_tensor("out", [S, E], fp16, kind="ExternalOutput")

    with tile.TileContext(nc) as tc, ExitStack() as ctx:
        persist = ctx.enter_context(tc.tile_pool(name="persist", bufs=1))
        xt_pool = ctx.enter_context(tc.tile_pool(name="xt", bufs=o["xt_bufs"]))
        expt_pool = ctx.enter_context(tc.tile_pool(name="expt",
                                                   bufs=o["expt_bufs"]))
        scr_pool = ctx.enter_context(tc.tile_pool(name="scr",
                                                  bufs=o["scr_bufs"]))
        acc_pool = ctx.enter_context(tc.tile_pool(name="acc",
                                                  bufs=o["acc_bufs"]))
        rinv_pool = ctx.enter_context(tc.tile_pool(name="rinv",
                                                   bufs=o["rinv_bufs"]))
        rs_pool = ctx.enter_context(tc.tile_pool(name="rs", bufs=o["rs_bufs"]))
        outst = ctx.enter_context(tc.tile_pool(name="outst",
                                               bufs=o["outst_bufs"]))
        ps_op = ctx.enter_context(tc.tile_pool(name="ps_op", bufs=o["op_bufs"],
                                               space="PSUM"))
        ps_cs = ctx.enter_context(tc.tile_pool(name="ps_cs", bufs=o["cs_bufs"],
                                               space="PSUM"))

        # ---- constants ----
        ones_col = persist.tile([128, 1], bf16, tag="ones_col")
        nc.vector.memset(ones_col, 1.0)
        # tri01[k, q] = 1 if q >= k else 0 (bf16) for the diagonal band
        tri01 = persist.tile([128, 128], bf16, tag="tri01")
        nc.gpsimd.memset(tri01, 1.0)
        nc.gpsimd.affine_select(out=tri01, in_=tri01,
                                compare_op=mybir.AluOpType.is_ge, fill=0.0,
                                base=0, pattern=[[1, 128]],
                                channel_multiplier=-1)

        # ---- persistent weights / biases (loaded once, resident) ----
        wq_sb = persist.tile([128, 8, C], bf16, tag="wq")
        wk_sb = persist.tile([128, 8, C], bf16, tag="wk")
        wv_sb = persist.tile([128, 8, C], bf16, tag="wv")
        wo_sb = persist.tile([128, HP, E], bf16, tag="wo")
        bqk = persist.tile([128, 4], f32, tag="bqk")
        bvt = persist.tile([128, HP], f32, tag="bvt")
        nc.sync.dma_start(out=wq_sb,
                          in_=wq_d.ap().rearrange("(c p) n -> p c n", p=128))
        nc.sync.dma_start(out=wk_sb,
                          in_=wk_d.ap().rearrange("(c p) n -> p c n", p=128))
        nc.sync.dma_start(out=wv_sb,
                          in_=wv_d.ap().rearrange("(c p) n -> p c n", p=128))
        nc.sync.dma_start(out=wo_sb,
                          in_=wo_d.ap().rearrange("(h p) n -> p h n", p=128))
        nc.sync.dma_start(out=bqk, in_=bqk_d.ap())
        nc.sync.dma_start(out=bvt, in_=bvt_d.ap())

        def make_op_emitter(ot_list):
            def emit_op_chunk(s, on_act=False):
                for nch in range(2):
                    nsl = slice(nch * 512, (nch + 1) * 512)
                    ps = ps_op.tile([128, 512], f32, tag="op")
                    for hh in range(HP):
                        nc.tensor.matmul(
                            ps,
                            lhsT=ot_list[hh][:, s * 128:(s + 1) * 128],
                            rhs=wo_sb[:, hh, nsl],
                            start=(hh == 0), stop=(hh == HP - 1))
                    osb = outst.tile([128, 512], fp16, tag="osb")
                    if on_act:
                        nc.scalar.activation(out=osb, in_=ps, func=AF.Copy)
                    else:
                        nc.vector.tensor_copy(osb, ps)
                    nc.sync.dma_start(
                        out=out_d[s * 128:(s + 1) * 128, nsl], in_=osb)
            return emit_op_chunk

        deferred = []   # (emitter, s) pairs carried into the next proj phase

        for it in range(n_iters):
            par = ("a" if it % 2 == 0 else "b") if o["dbuf"] else "a"
            qt_sb = [persist.tile([128, S], bf16, tag=f"qt{m}{par}",
                                  name=f"qt{m}{par}")
                     for m in range(HP)]
            kt_sb = [persist.tile([128, S], bf16, tag=f"kt{m}{par}",
                                  name=f"kt{m}{par}")
                     for m in range(HP)]
            v_sb = persist.tile([128, KT, C], bf16, tag=f"v{par}", name=f"v{par}")
            ot_sb = [persist.tile([128, S], bf16, tag=f"ot{m}{par}",
                                  name=f"ot{m}{par}")
                     for m in range(HP)]

            # ================= Phase 1: projections =================
            # Deferred out-proj chunks from the previous iteration run here,
            # interleaved into the QT matmul stream to fill DMA-wait slack.
            dq = list(deferred)
            deferred = []
            with tc.tile_pool(name="ps_proj", bufs=o["proj_bufs"],
                              space="PSUM") as ps_proj:
                # QT / KT: [C, S] = W.T @ X.T; k-chunk outer, one head-pass
                # at a time (4 PSUM banks each).
                for tname, xdram, wsb, qkts, bcol in (
                    ("q", xq_t, wq_sb, qt_sb, 0),
                    ("k", xk_t, wk_sb, kt_sb, 2),
                ):
                    xcs = []
                    for c in range(8):
                        xc = xt_pool.tile([128, S], bf16, tag="xtc")
                        nc.sync.dma_start(
                            out=xc, in_=xdram[c * 128:(c + 1) * 128, :])
                        xcs.append(xc)
                    for m in range(HP):
                        pss = [ps_proj.tile([128, 512], f32, tag="ps_proj",
                                            name=f"ps_{tname}{m}{_i}")
                               for _i in range(QC)]
                        for c in range(8):
                            for n in range(QC):
                                nc.tensor.matmul(
                                    pss[n],
                                    lhsT=wsb[:, c, m * 128:(m + 1) * 128],
                                    rhs=xcs[c][:, n * 512:(n + 1) * 512],
                                    start=(c == 0), stop=(c == 7))
                            if m == 0 and dq and (
                                    tname == "q" or o.get("defer_more")):
                                em, s_ = dq.pop(0)
                                em(s_)
                        for n in range(QC):
                            nc.scalar.activation(
                                out=qkts[m][:, n * 512:(n + 1) * 512],
                                in_=pss[n],
                                func=AF.Identity,
                                bias=bqk[:, bcol + m:bcol + m + 1], scale=1.0)
                while dq:
                    em, s_ = dq.pop(0)
                    em(s_)

                # V natural: [S, C] = X @ Wv (lhsT = XT chunk slice); psum ->
                # SBUF copy on the Act engine (no bias: bv applied later).
                xcs = []
                for c in range(8):
                    xc = xt_pool.tile([128, S], bf16, tag="xtc")
                    nc.sync.dma_start(out=xc, in_=xv_t[c * 128:(c + 1) * 128, :])
                    xcs.append(xc)
                for s in range(KT):
                    ps = ps_proj.tile([128, C], f32, tag="ps_proj")
                    for c in range(8):
                        nc.tensor.matmul(
                            ps,
                            lhsT=xcs[c][:, s * 128:(s + 1) * 128],
                            rhs=wv_sb[:, c, :],
                            start=(c == 0), stop=(c == 7))
                    nc.scalar.activation(out=v_sb[:, s, :], in_=ps,
                                         func=AF.Copy)

            # ================= Phase 2: attention + fused out-proj ========
            with tc.tile_pool(name="ps_sc", bufs=o["sc_bufs"],
                              space="PSUM") as ps_sc, \
                 tc.tile_pool(name="ps_ot", bufs=o["ot_bufs"],
                              space="PSUM") as ps_ot:

                emit_op_chunk = make_op_emitter(ot_sb)

                # big-j first; tiny-j steps are sandwiched between large-PE
                # steps so their softmax chains hide under PE work.
                HJ = [(0, 3), (1, 3), (0, 0), (0, 2), (1, 2), (1, 0),
                      (0, 1), (1, 1)]
                # out-proj schedule (2 seq-tiles per step, one group behind);
                # groups not ready until late are deferred into the next
                # iteration's projection phase.
                if o.get("defer_more"):
                    OPS = {5: [8, 9], 6: [10, 11], 7: [0, 1]}
                    DEFER = [12, 13, 14, 15, 2, 3, 4, 5, 6, 7]
                else:
                    OPS = {3: [12, 13], 4: [14, 15], 5: [8, 9], 6: [10, 11],
                           7: [0, 1]}
                    DEFER = [2, 3, 4, 5, 6, 7]

                ET = {}
                OFFP = {}

                def sc_block(i):
                    """Scores + exp + masking for step i (PE/Act/Pool side;
                    denominator partials on DVE as a running accumulate)."""
                    h, j = HJ[i]
                    qsl = slice(j * 512, (j + 1) * 512)
                    et = expt_pool.tile([128, KT, 512], bf16, tag="et",
                                        name=f"et{i}")
                    ET[i] = et
                    nc.gpsimd.memset(et[:, 4 * j + 1:4 * j + 4, :], 0.0)

                    offp = []
                    for p in range(2 * j):
                        ps2 = ps_sc.tile([128, 2, 512], f32, tag="sc2",
                                         name=f"sc{i}_{p}")
                        for u in range(2):
                            kti = 2 * p + u
                            nc.tensor.matmul(
                                ps2[:, u, :],
                                lhsT=kt_sb[h][:, kti * 128:(kti + 1) * 128],
                                rhs=qt_sb[h][:, qsl],
                                start=True, stop=True)
                        nc.scalar.activation(
                            out=et[:, 2 * p:2 * p + 2, :], in_=ps2,
                            func=AF.Exp, scale=SCALE)
                        if p % 2 == 1:
                            t4 = scr_pool.tile([128, 4, 512], bf16, tag="scr",
                                               name=f"scr{i}_{p}")
                            nc.vector.tensor_add(
                                t4[:, 0:2, :], et[:, 2 * p - 2:2 * p, :],
                                et[:, 2 * p:2 * p + 2, :])
                            nc.vector.tensor_add(t4[:, 0, :], t4[:, 0, :],
                                                 t4[:, 1, :])
                            offp.append(t4)
                    OFFP[i] = offp

                    band_eng = nc.gpsimd if o.get("lookahead") else nc.vector
                    for dp in range(2):
                        ps2 = ps_sc.tile([128, 2, 512], f32, tag="sc2",
                                         name=f"scd{i}_{dp}")
                        for u in range(2):
                            t = 2 * dp + u
                            w0 = 128 * t
                            nc.tensor.matmul(
                                ps2[:, u, w0:512],
                                lhsT=kt_sb[h][:, (4 * j + t) * 128:
                                              (4 * j + t + 1) * 128],
                                rhs=qt_sb[h][:, j * 512 + w0:(j + 1) * 512],
                                start=True, stop=True)
                        for u in range(2):
                            t = 2 * dp + u
                            w0 = 128 * t
                            nc.scalar.activation(
                                out=et[:, 4 * j + t, w0:512],
                                in_=ps2[:, u, w0:512],
                                func=AF.Exp, scale=SCALE)
                        for u in range(2):
                            t = 2 * dp + u
                            w0 = 128 * t
                            band_eng.tensor_mul(
                                et[:, 4 * j + t, w0:w0 + 128],
                                et[:, 4 * j + t, w0:w0 + 128], tri01)

                def rest_block(i):
                    """attn@V + softmax chain + scheduled out-proj chunks."""
                    h, j = HJ[i]
                    nk = 4 * (j + 1)
                    qsl = slice(j * 512, (j + 1) * 512)
                    et = ET[i]
                    offp = OFFP[i]

                    ot = ps_ot.tile([128, 512], f32, tag="ot", name=f"ot{i}")
                    for kti in range(nk):
                        w0 = 128 * (kti - 4 * j) if kti >= 4 * j else 0
                        nc.tensor.matmul(
                            ot[:, w0:512],
                            lhsT=v_sb[:, kti, h * 128:(h + 1) * 128],
                            rhs=et[:, kti, w0:512],
                            start=(kti == 0), stop=(kti == nk - 1))

                    accum = acc_pool.tile([128, 512], bf16, tag="acc",
                                          name=f"acc{i}")
                    dt = scr_pool.tile([128, 2, 512], bf16, tag="scrd",
                                       name=f"dt{i}")
                    nc.vector.tensor_add(dt, et[:, 4 * j:4 * j + 2, :],
                                         et[:, 4 * j + 2:4 * j + 4, :])
                    if not offp:
                        nc.vector.tensor_add(accum, dt[:, 0, :], dt[:, 1, :])
                    else:
                        nc.vector.tensor_add(dt[:, 0, :], dt[:, 0, :],
                                             dt[:, 1, :])
                        if len(offp) == 1:
                            nc.vector.tensor_add(accum, dt[:, 0, :],
                                                 offp[0][:, 0, :])
                        elif len(offp) == 2:
                            nc.vector.tensor_add(dt[:, 1, :],
                                                 offp[0][:, 0, :],
                                                 offp[1][:, 0, :])
                            nc.vector.tensor_add(accum, dt[:, 0, :],
                                                 dt[:, 1, :])
                        else:
                            nc.vector.tensor_add(dt[:, 1, :],
                                                 offp[0][:, 0, :],
                                                 offp[1][:, 0, :])
                            nc.vector.tensor_add(dt[:, 0, :], dt[:, 0, :],
                                                 offp[2][:, 0, :])
                            nc.vector.tensor_add(accum, dt[:, 0, :],
                                                 dt[:, 1, :])

                    rs = rs_pool.tile([128, 512], f32, tag="rs",
                                      name=f"rs{i}")
                    if o["use_parred"]:
                        nc.gpsimd.partition_all_reduce(
                            rs, accum, channels=128,
                            reduce_op=bass_isa.ReduceOp.add)
                        nc.vector.reciprocal(rs, rs)
                    else:
                        cs = ps_cs.tile([1, 512], f32, tag="cs",
                                        name=f"cs{i}")
                        nc.tensor.matmul(cs, lhsT=ones_col, rhs=accum,
                                         start=True, stop=True)
                        rin = rinv_pool.tile([1, 512], f32, tag="rinv",
                                             name=f"rin{i}")
                        nc.vector.reciprocal(rin, cs)
                        nc.gpsimd.partition_broadcast(rs, rin)
                    nc.vector.tensor_mul(ot_sb[h][:, qsl], ot, rs)
                    nc.vector.tensor_scalar_add(ot_sb[h][:, qsl],
                                                ot_sb[h][:, qsl],
                                                bvt[:, h:h + 1])

                    for s in OPS.get(i, []):
                        emit_op_chunk(s)

                if o.get("lookahead"):
                    sc_block(0)
                    for i in range(len(HJ)):
                        if i + 1 < len(HJ):
                            sc_block(i + 1)
                        rest_block(i)
                else:
                    for i in range(len(HJ)):
                        sc_block(i)
                        rest_block(i)

                for s in DEFER:
                    deferred.append((emit_op_chunk, s))

        # drain any deferred out-proj chunks of the final iteration
        for em, s_ in deferred:
            em(s_)

    nc.compile()
    return nc


def get_program(n_iters: int = 1):
    if n_iters not in _prog_cache:
        _prog_cache[n_iters] = build_program(n_iters)
    return _prog_cache[n_iters]


def make_in_maps(query, key_, value, Wq, bq, Wk, bk, Wv, bv, Wo, bo, mask):
    """Host-side sharding: build the 8 per-core input maps."""
    query = np.asarray(query, np.float32)
    key_ = np.asarray(key_, np.float32)
    value = np.asarray(value, np.float32)

    # transposed bf16 activations per batch: [E, S]
    xt = {}
    for b in range(B):
        xt[("q", b)] = np.ascontiguousarray(query[b].T.astype(BF16))
        xt[("k", b)] = np.ascontiguousarray(key_[b].T.astype(BF16))
        xt[("v", b)] = np.ascontiguousarray(value[b].T.astype(BF16))

    Wq = np.asarray(Wq, np.float32)
    Wk = np.asarray(Wk, np.float32)
    Wv = np.asarray(Wv, np.float32)
    Wo = np.asarray(Wo, np.float32)
    bq = np.asarray(bq, np.float32)
    bk = np.asarray(bk, np.float32)
    bv = np.asarray(bv, np.float32)

    in_maps = []
    for c in range(NCORES):
        b, g = divmod(c, 4)
        c0 = C * g
        bqk = np.stack([bq[c0:c0 + 128], bq[c0 + 128:c0 + 256],
                        bk[c0:c0 + 128], bk[c0 + 128:c0 + 256]], axis=1)
        bvt = np.stack([bv[c0:c0 + 128], bv[c0 + 128:c0 + 256]], axis=1)
        in_maps.append({
            "xq_t": xt[("q", b)],
            "xk_t": xt[("k", b)],
            "xv_t": xt[("v", b)],
            "wq": Wq[:, c0:c0 + C].astype(BF16),
            "wk": Wk[:, c0:c0 + C].astype(BF16),
            "wv": Wv[:, c0:c0 + C].astype(BF16),
            "wo": np.ascontiguousarray(Wo[c0:c0 + C, :]).astype(BF16),
            "bqk": np.ascontiguousarray(bqk, dtype=np.float32),
            "bvt": np.ascontiguousarray(bvt, dtype=np.float32),
        })
    return in_maps


_bo_cache = {}


def gather_output(results, bo):
    out = np.zeros((B, S, E), np.float32)
    for c in range(NCORES):
        b = c // 4
        out[b] += results[c]["out"].astype(np.float32)
    out += np.asarray(bo, np.float32)
    return out


def kernel(**inputs) -> np.ndarray:
    from concourse.bass_utils import run_bass_kernel_spmd

    nc = get_program(1)
    in_maps = make_in_maps(**inputs)
    res = run_bass_kernel_spmd(nc, in_maps, core_ids=list(range(NCORES)))
    return gather_output(res.results, inputs["bo"])
